# revision 12
# baseline (speedup 1.0000x reference)
"""Multi-head attention on 8 trn2 NeuronCores, head-parallel (2 heads/core).

Math per head h (reference semantics):
  Q = query @ Wq[h] + bq[h];  K = key @ Wk[h] (+ bk[h] dropped: constant
  along the softmax axis, provably softmax-invariant);  V = query @ Wv[h]
  P = exp(Q K^T / sqrt(D));  alpha = P / rowsum(P)
  ctx = alpha @ V;  y_h = (ctx @ Wp[h] + bp[h]) @ Wo[h]
  out = sum_h y_h + bo

Device-side formulation (transposed layouts, fp16 storage, f32 PSUM):
  Host pre-packs query/key into [B, NTB, 128, NCH, TB] so every x-chunk DMA
  is fully contiguous (8 KiB/partition lines), and pre-packs weights into
  their SBUF-resident layouts. Per core: project QT/KT (K without bias) and
  Vn per head, attention with unnormalized softmax (rowsum via ones-matmul
  of the vector-accumulated exp tiles, normalization folded after PV),
  y^T[e, tok] partial = sum_{h} W_h^T @ ctxn_h + bias via vector add, where
  W_h = Wp[h] @ Wo[h] (host-premultiplied) and bias collects bv/bp/bo terms.
  Per-qblock fp16 ReduceScatter straight into the output tensor; host
  reassembles and casts to f32.
"""

import sys

if "/opt/trn_rl_repo" not in sys.path:
    sys.path.insert(0, "/opt/trn_rl_repo")

import ml_dtypes
import numpy as np

import concourse.mybir as mybir
import concourse.tile as tile
from concourse import bacc
from concourse.bass_utils import run_bass_kernel_spmd

B, S = 4, 2048
IN, D, H = 1024, 128, 16
NCORES = 8
HPC = H // NCORES  # heads per core
NCH = IN // 128  # input chunks
TB = 512  # projection token block
NTB = S // TB
QB = 512  # attention query block
NQB = S // QB
KT = 128  # attention key tile
NKT = S // KT
ESH = D // NCORES  # output shard rows per core

f32 = mybir.dt.float32
f16 = mybir.dt.float16
AF = mybir.ActivationFunctionType

_cache = {}


def build():
    nc = bacc.Bacc(None, target_bir_lowering=False, num_devices=NCORES)

    qp = nc.dram_tensor("qp", [B, NTB, 128, NCH, TB], f16, kind="ExternalInput")
    kp = nc.dram_tensor("kp", [B, NTB, 128, NCH, TB], f16, kind="ExternalInput")
    wq = nc.dram_tensor("wq", [128, HPC, NCH, D], f16, kind="ExternalInput")
    wk = nc.dram_tensor("wk", [128, HPC, NCH, D], f16, kind="ExternalInput")
    wv = nc.dram_tensor("wv", [128, HPC, NCH, D], f16, kind="ExternalInput")
    wh = nc.dram_tensor("wh", [128, HPC, D], f16, kind="ExternalInput")
    bqT = nc.dram_tensor("bqT", [D, HPC], f32, kind="ExternalInput")
    biasv = nc.dram_tensor("biasv", [D, 1], f32, kind="ExternalInput")
    onemb = nc.dram_tensor("onemb", [D, D], f16, kind="ExternalInput")

    out_y = nc.dram_tensor("out_y", [B, NQB, ESH, QB], f16, kind="ExternalOutput")
    # RS has a large fixed per-op cost and serializes on the CC path: use
    # per-qbp chunks (fewer ops) for batches 0..B-2, per-qblock chunks for
    # the last batch so the final exposed op is small.
    y_bounce = [
        [nc.dram_tensor(f"y_bounce{b}_{q}", [D, 2 * QB], f16) for q in range(NQB // 2)]
        for b in range(B)
    ]
    y_shard = [
        [nc.dram_tensor(f"y_shard{b}_{q}", [ESH, 2 * QB], f16) for q in range(NQB // 2)]
        for b in range(B)
    ]

    scale = 1.0 / float(np.sqrt(D))

    with tile.TileContext(nc) as tc:
        with (
            tc.tile_pool(name="const", bufs=1) as cpool,
            tc.tile_pool(name="xch", bufs=24) as xch,
            tc.tile_pool(name="qkv", bufs=2) as qkv,
            tc.tile_pool(name="work", bufs=2) as work,
            tc.tile_pool(name="pexpp", bufs=4) as pexpp,
            tc.tile_pool(name="ps", bufs=2, space="PSUM") as ps,
        ):
            # ---- resident constants (all pre-packed host-side: contiguous DMA)
            # on the scalar ring so the x-chunk stream (sync ring) does not
            # queue behind them at startup ----
            wq_sb = cpool.tile([128, HPC, NCH, D], f16, tag="wq_sb")
            wk_sb = cpool.tile([128, HPC, NCH, D], f16, tag="wk_sb")
            wv_sb = cpool.tile([128, HPC, NCH, D], f16, tag="wv_sb")
            for sb_t, dram_t in ((wq_sb, wq), (wk_sb, wk), (wv_sb, wv)):
                nc.scalar.dma_start(sb_t[:], dram_t[:])
            wh_sb = cpool.tile([128, HPC, D], f16, tag="wh_sb")
            nc.scalar.dma_start(wh_sb[:], wh[:])
            bq_sb = cpool.tile([128, HPC], f32, tag="bq_sb")
            nc.scalar.dma_start(bq_sb[:], bqT[:])
            biasv_sb = cpool.tile([128, 1], f32, tag="biasv_sb")
            nc.scalar.dma_start(biasv_sb[:], biasv[:])
            onemb_sb = cpool.tile([D, D], f16, tag="onemb_sb")
            nc.scalar.dma_start(onemb_sb[:], onemb[:])

            QTd, KTd, Vnd = {}, {}, {}

            def proj_batch(b):
                # ---- projections: Q & V from qp, K from kp ----
                QT = QTd[b] = [qkv.tile([128, S], f16, tag=f"QT{h}", name=f"QT{h}") for h in range(HPC)]
                KTs = KTd[b] = [qkv.tile([128, S], f16, tag=f"KT{h}", name=f"KT{h}") for h in range(HPC)]
                Vn = Vnd[b] = [qkv.tile([128, S], f16, tag=f"VN{h}", name=f"VN{h}") for h in range(HPC)]

                for tb in range(NTB):
                    sl = slice(tb * TB, (tb + 1) * TB)
                    chs = xch.tile([128, NCH, TB], f16, tag="xch", bufs=6)
                    nc.sync.dma_start(chs[:], qp[b, tb])
                    pq = ps.tile([128, 2 * TB], f32, tag="pS", name="pq", bufs=2)
                    for h in range(HPC):
                        for c in range(NCH):
                            nc.tensor.matmul(
                                pq[:, h * TB : (h + 1) * TB],
                                wq_sb[:, h, c, :], chs[:, c, :],
                                start=(c == 0), stop=(c == NCH - 1),
                            )
                    for h in range(HPC):
                        with nc.allow_low_precision(reason="fp16 PE operand"):
                            nc.vector.tensor_scalar_add(
                                QT[h][:, sl], pq[:, h * TB : (h + 1) * TB],
                                bq_sb[:, h : h + 1],
                            )
                    # V in natural [tok, d] layout: chunk subtiles as stationary
                    for t in range(TB // 128):
                        pvt = ps.tile([128, 2 * D], f32, tag="pC", name="pvt", bufs=4)
                        for c in range(NCH):
                            nc.tensor.matmul(
                                pvt[:],
                                chs[:, c, t * 128 : (t + 1) * 128],
                                wv_sb[:, :, c, :],
                                start=(c == 0), stop=(c == NCH - 1),
                            )
                        col = tb * TB + t * 128
                        for h in range(HPC):
                            with nc.allow_low_precision(reason="fp16 PV operand"):
                                nc.vector.tensor_copy(
                                    Vn[h][:, col : col + 128],
                                    pvt[:, h * D : (h + 1) * D],
                                )

                for tb in range(NTB):
                    sl = slice(tb * TB, (tb + 1) * TB)
                    chs = xch.tile([128, NCH, TB], f16, tag="xch", bufs=6)
                    nc.sync.dma_start(chs[:], kp[b, tb])
                    pk = ps.tile([128, 2 * TB], f32, tag="pS", name="pk", bufs=2)
                    for h in range(HPC):
                        for c in range(NCH):
                            nc.tensor.matmul(
                                pk[:, h * TB : (h + 1) * TB],
                                wk_sb[:, h, c, :], chs[:, c, :],
                                start=(c == 0), stop=(c == NCH - 1),
                            )
                    for h in range(HPC):
                        with nc.allow_low_precision(reason="fp16 PE operand"):
                            nc.vector.tensor_copy(
                                KTs[h][:, sl], pk[:, h * TB : (h + 1) * TB],
                            )

            def attn_batch(b):
                QT, KTs, Vn = QTd.pop(b), KTd.pop(b), Vnd.pop(b)
                # ---- attention: qblock pairs share 2-bank psum + one wide exp ----
                for qbp in range(NQB // 2):
                    q0 = qbp * 2 * QB
                    sl0 = slice(q0, q0 + QB)
                    sl1 = slice(q0 + QB, q0 + 2 * QB)
                    ctxns = []
                    for h in range(HPC):
                        pctx0 = ps.tile([128, QB], f32, tag="pC", name="pctx0", bufs=4)
                        pctx1 = ps.tile([128, QB], f32, tag="pC", name="pctx1", bufs=4)
                        acc_d = work.tile([128, 2 * QB], f16, tag="acc_d", name="acc_d")
                        st = [True, None]
                        # 2-kt supersteps: the two PV matmuls into the same
                        # psum bank run back-to-back (same accumulation
                        # group), which keeps the PE at pure stream rate
                        # (~213ns/512col) instead of paying the ~50ns
                        # group-switch penalty per matmul.
                        for kt2 in range(NKT // 2):
                            kta, ktb = 2 * kt2, 2 * kt2 + 1
                            ksla = slice(kta * 128, (kta + 1) * 128)
                            kslb = slice(ktb * 128, (ktb + 1) * 128)
                            ps2a = ps.tile([128, 2 * QB], f32, tag="pS", name="ps2a", bufs=2)
                            ps2b = ps.tile([128, 2 * QB], f32, tag="pS", name="ps2b", bufs=2)
                            nc.tensor.matmul(
                                ps2a[:, :QB], KTs[h][:, ksla], QT[h][:, sl0],
                                start=True, stop=True,
                            )
                            nc.tensor.matmul(
                                ps2a[:, QB:], KTs[h][:, ksla], QT[h][:, sl1],
                                start=True, stop=True,
                            )
                            nc.tensor.matmul(
                                ps2b[:, :QB], KTs[h][:, kslb], QT[h][:, sl0],
                                start=True, stop=True,
                            )
                            nc.tensor.matmul(
                                ps2b[:, QB:], KTs[h][:, kslb], QT[h][:, sl1],
                                start=True, stop=True,
                            )
                            pexpa = pexpp.tile([128, 2 * QB], f16, tag="pexp", bufs=8)
                            pexpb = pexpp.tile([128, 2 * QB], f16, tag="pexp", bufs=8)
                            nc.scalar.activation(pexpa[:], ps2a[:], AF.Exp, scale=scale)
                            nc.scalar.activation(pexpb[:], ps2b[:], AF.Exp, scale=scale)
                            nc.tensor.matmul(
                                pctx0[:], Vn[h][:, ksla], pexpa[:, :QB],
                                start=(kta == 0), stop=False,
                            )
                            nc.tensor.matmul(
                                pctx0[:], Vn[h][:, kslb], pexpb[:, :QB],
                                start=False, stop=(ktb == NKT - 1),
                            )
                            nc.tensor.matmul(
                                pctx1[:], Vn[h][:, ksla], pexpa[:, QB:],
                                start=(kta == 0), stop=False,
                            )
                            nc.tensor.matmul(
                                pctx1[:], Vn[h][:, kslb], pexpb[:, QB:],
                                start=False, stop=(ktb == NKT - 1),
                            )
                            with nc.allow_low_precision(reason="fp16 rowsum acc"):
                                for pexp in (pexpa, pexpb):
                                    if st[0] and st[1] is None:
                                        st[1] = pexp
                                    elif st[0]:
                                        nc.vector.tensor_add(acc_d[:], st[1][:], pexp[:])
                                        st[0] = False
                                    else:
                                        nc.vector.tensor_add(acc_d[:], acc_d[:], pexp[:])
                        # norm for this head right away: hides under the
                        # other head's kt loop; rowsum outputs use the pC
                        # slots the pctx pair is about to free
                        pbc0 = ps.tile([128, QB], f32, tag="pC", name="pbc0", bufs=4)
                        pbc1 = ps.tile([128, QB], f32, tag="pC", name="pbc1", bufs=4)
                        nc.tensor.matmul(
                            pbc0[:], onemb_sb[:], acc_d[:, :QB], start=True, stop=True
                        )
                        nc.tensor.matmul(
                            pbc1[:], onemb_sb[:], acc_d[:, QB:], start=True, stop=True
                        )
                        rsbr = work.tile([128, 2 * QB], f32, tag="rsbr", name="rsbr", bufs=2)
                        nc.vector.reciprocal_approx_fast(out=rsbr[:, :QB], in_=pbc0[:])
                        nc.vector.reciprocal_approx_fast(out=rsbr[:, QB:], in_=pbc1[:])
                        ctxn = work.tile([128, 2 * QB], f16, tag="ctxn", name="ctxn")
                        with nc.allow_low_precision(reason="fp16 PE operand"):
                            nc.vector.tensor_mul(ctxn[:, :QB], pctx0[:], rsbr[:, :QB])
                            nc.vector.tensor_mul(ctxn[:, QB:], pctx1[:], rsbr[:, QB:])
                        ctxns.append(ctxn)

                    pzs = [
                        ps.tile([128, QB], f32, tag="pC", name="pz", bufs=4)
                        for _ in range(2)
                    ]
                    for h in range(HPC):
                        nc.tensor.matmul(
                            pzs[0][:], wh_sb[:, h, :], ctxns[h][:, :QB],
                            start=(h == 0), stop=(h == HPC - 1),
                        )
                        nc.tensor.matmul(
                            pzs[1][:], wh_sb[:, h, :], ctxns[h][:, QB:],
                            start=(h == 0), stop=(h == HPC - 1),
                        )
                    for half in range(2):
                        ytile = work.tile([128, QB], f16, tag="ytile")
                        with nc.allow_low_precision(reason="fp16 partial y"):
                            nc.vector.tensor_scalar_add(
                                ytile[:], pzs[half][:], biasv_sb[:, 0:1]
                            )
                        nc.gpsimd.dma_start(
                            y_bounce[b][qbp][:, half * QB : (half + 1) * QB],
                            ytile[:],
                        )
                    nc.gpsimd.collective_compute(
                        "ReduceScatter",
                        mybir.AluOpType.add,
                        replica_groups=[list(range(NCORES))],
                        ins=[y_bounce[b][qbp][:].opt()],
                        outs=[y_shard[b][qbp][:].opt()],
                    )
                    nc.sync.dma_start(
                        out_y[b, 2 * qbp : 2 * qbp + 2],
                        y_shard[b][qbp][:].rearrange("e (q n) -> q e n", q=2),
                    )

            for b in range(B):
                proj_batch(b)
                if b > 0:
                    attn_batch(b - 1)
            attn_batch(B - 1)

    nc.compile()
    return nc


def kernel(**inputs):
    query = np.asarray(inputs["query"], np.float32)
    key = np.asarray(inputs["key"], np.float32)
    Wq, bq = np.asarray(inputs["Wq"], np.float32), np.asarray(inputs["bq"], np.float32)
    Wk = np.asarray(inputs["Wk"], np.float32)
    Wv, bv = np.asarray(inputs["Wv"], np.float32), np.asarray(inputs["bv"], np.float32)
    Wp, bp = np.asarray(inputs["Wp"], np.float32), np.asarray(inputs["bp"], np.float32)
    Wo, bo = np.asarray(inputs["Wo"], np.float32), np.asarray(inputs["bo"], np.float32)

    f16h = ml_dtypes.float16 if hasattr(ml_dtypes, "float16") else np.float16
    # [B, NTB, 128, NCH, TB]: fully contiguous per-(b,tb) x-chunk blocks
    qp = np.ascontiguousarray(
        query.reshape(B, NTB, TB, NCH, 128).transpose(0, 1, 4, 3, 2)
    ).astype(f16h)
    kp = np.ascontiguousarray(
        key.reshape(B, NTB, TB, NCH, 128).transpose(0, 1, 4, 3, 2)
    ).astype(f16h)

    if "nc" not in _cache:
        _cache["nc"] = build()
    nc = _cache["nc"]

    def pack_w(Wfull, hs):
        # [HPC, IN, D] -> [128, HPC, NCH, D]
        return np.ascontiguousarray(
            Wfull[hs].reshape(HPC, NCH, 128, D).transpose(2, 0, 1, 3)
        ).astype(f16h)

    in_maps = []
    for i in range(NCORES):
        hs = slice(i * HPC, (i + 1) * HPC)
        Wo_h = Wo.reshape(H, D, D)  # rows of Wo per head
        wh = np.einsum(
            "hde,hef->hdf",
            Wp[hs].astype(np.float64),
            Wo_h[hs].astype(np.float64),
        ).astype(np.float32)
        bias = (
            np.einsum("hd,hdf->f", bv[hs].astype(np.float64), wh.astype(np.float64))
            + np.einsum(
                "hd,hdf->f", bp[hs].astype(np.float64), Wo_h[hs].astype(np.float64)
            )
            + bo.astype(np.float64) / NCORES
        ).astype(np.float32)
        in_maps.append(
            {
                "qp": qp,
                "kp": kp,
                "wq": pack_w(Wq, hs),
                "wk": pack_w(Wk, hs),
                "wv": pack_w(Wv, hs),
                "wh": np.ascontiguousarray(wh.transpose(1, 0, 2)).astype(f16h),
                "bqT": np.ascontiguousarray(bq[hs].T),
                "biasv": bias.reshape(D, 1),
                "onemb": np.ones((D, D), f16h),
            }
        )

    res = run_bass_kernel_spmd(nc, in_maps, core_ids=list(range(NCORES)))
    _cache["last_result"] = res
    # out_y core i: [B, NQB, ESH, QB] = y^T rows [i*ESH:(i+1)*ESH]
    arr = np.stack([np.asarray(res.results[i]["out_y"]) for i in range(NCORES)])
    # [NC, B, NQB, ESH, QB] -> [B, NQB, QB, NC, ESH] -> [B, S, D]
    y = arr.transpose(1, 2, 4, 0, 3).reshape(B, S, D)
    return np.ascontiguousarray(y).astype(np.float32)
